# revision 5
# baseline (speedup 1.0000x reference)
"""AF-LSTM Trainium2 kernel: 8-way batch-parallel, no collectives.

Per core (8 batch rows): gather embeddings, LSTM recurrence in transposed
(gate-major) layout, circular-correlation attention via per-batch circulant
matmul folded into W_y, final MLP + softmax. Host concatenates per-core
[8,256] outputs.
"""

import numpy as np
import ml_dtypes

import concourse.bacc as bacc
import concourse.tile as tile
from concourse import bass, mybir
from concourse.bass import IndirectOffsetOnAxis
from concourse.bass_utils import run_bass_kernel_spmd
from concourse.masks import make_identity

F32 = mybir.dt.float32
BF16 = mybir.dt.bfloat16
I32 = mybir.dt.int32
AF = mybir.ActivationFunctionType
ALU = mybir.AluOpType

V, D, B = 50000, 256, 64
NCORES, BL = 8, 8
G4 = 4 * D
bf16 = ml_dtypes.bfloat16

# gate blocks of 128 rows reordered to [i0,i1,f0,f1,o0,o1,g0,g1]
_PERM = np.concatenate([
    np.arange(0, 256),        # i
    np.arange(256, 512),      # f
    np.arange(768, 1024),     # o
    np.arange(512, 768),      # g
])


def build(T_steps=512):
    nc = bacc.Bacc(None, target_bir_lowering=False)
    NT = T_steps * BL // 128          # gather tiles of 128 tokens
    NCH = T_steps * BL // 512         # 512-token chunks for xs matmul

    emb_e = nc.declare_dram_parameter("emb", [V, D], F32, isOutput=False)
    xp_e = nc.declare_dram_parameter("xp", [128, NT], I32, isOutput=False)
    sidx_e = nc.declare_dram_parameter("sidx", [64, 1], I32, isOutput=False)
    wihT_e = nc.declare_dram_parameter("wihT", [128, 2048], BF16, isOutput=False)
    whhT_e = nc.declare_dram_parameter("whhT", [128, 2048], BF16, isOutput=False)
    bl_e = nc.declare_dram_parameter("bl", [128, 8], F32, isOutput=False)
    wy_e = nc.declare_dram_parameter("wy", [128, 512], BF16, isOutput=False)
    wtoh_e = nc.declare_dram_parameter("wtoh", [128, 128], BF16, isOutput=False)
    sel_e = nc.declare_dram_parameter("sel", [8, 1024], BF16, isOutput=False)
    wpT_e = nc.declare_dram_parameter("wpT", [128, 512], BF16, isOutput=False)
    wxT_e = nc.declare_dram_parameter("wxT", [128, 512], BF16, isOutput=False)
    wfT_e = nc.declare_dram_parameter("wfT", [128, 512], BF16, isOutput=False)
    bf_e = nc.declare_dram_parameter("bf", [128, 2], F32, isOutput=False)
    out_e = nc.declare_dram_parameter("out", [8, 256], F32, isOutput=True)
    s2d = nc.dram_tensor("s2d", [8, 512], BF16)

    with tile.TileContext(nc) as tc:
        with (
            tc.tile_pool(name="const", bufs=1) as cp,
            tc.tile_pool(name="big", bufs=1) as bigp,
        ):
            # ---- constants / weights to SBUF ----
            xp_sb = cp.tile([128, NT], I32)
            sidx_sb = cp.tile([64, 1], I32)
            wihT_sb = cp.tile([128, 2048], BF16)
            whhT_sb = cp.tile([128, 2048], BF16)
            bl_sb = cp.tile([128, 8], F32)
            wy_sb = cp.tile([128, 512], BF16)
            wtoh_sb = cp.tile([128, 128], BF16)
            sel_sb = cp.tile([8, 1024], BF16)
            wpT_sb = cp.tile([128, 512], BF16)
            wxT_sb = cp.tile([128, 512], BF16)
            wfT_sb = cp.tile([128, 512], BF16)
            bf_sb = cp.tile([128, 2], F32)
            for dst, src in [(xp_sb, xp_e), (sidx_sb, sidx_e), (wihT_sb, wihT_e),
                             (whhT_sb, whhT_e), (bl_sb, bl_e), (wy_sb, wy_e),
                             (wtoh_sb, wtoh_e), (sel_sb, sel_e), (wpT_sb, wpT_e),
                             (wxT_sb, wxT_e), (wfT_sb, wfT_e), (bf_sb, bf_e)]:
                nc.sync.dma_start(dst[:], src[:])
            ident = cp.tile([128, 128], F32)
            make_identity(nc, ident[:])
            ident_bf = cp.tile([128, 128], BF16)
            nc.vector.tensor_copy(ident_bf[:], ident[:])
            ones64 = cp.tile([64, 1], F32)
            nc.gpsimd.memset(ones64[:], 1.0)
            ones1w = cp.tile([1, 128], F32)
            nc.gpsimd.memset(ones1w[:], 1.0)
            ones128 = cp.tile([128, 1], F32)
            nc.gpsimd.memset(ones128[:], 1.0)

            # ---- persistent big tensors ----
            eT0 = bigp.tile([128, T_steps * 8], BF16)
            eT1 = bigp.tile([128, T_steps * 8], BF16)
            xsT = bigp.tile([128, T_steps * 64], BF16)
            hT = bigp.tile([128, (T_steps + 1) * 16], BF16)
            s2_sb = bigp.tile([8, 512], BF16)
            rT_sb = bigp.tile([128, 16], F32)
            rT_bf = bigp.tile([128, 16], BF16)
            a_sb = bigp.tile([8, T_steps], BF16)

            # ================= s-branch (batchnormed aspect embedding) ======
            with (
                tc.tile_pool(name="swork", bufs=1) as sw,
                tc.tile_pool(name="spsum", bufs=1, space="PSUM") as sps,
            ):
                semb = sw.tile([64, 256], F32)
                nc.gpsimd.indirect_dma_start(
                    out=semb[:], out_offset=None, in_=emb_e[:],
                    in_offset=IndirectOffsetOnAxis(ap=sidx_sb[:, :1], axis=0))
                mu_ps = sps.tile([1, 256], F32, space="PSUM")
                nc.tensor.matmul(mu_ps[:], ones64[:], semb[:], start=True, stop=True)
                mu = sw.tile([1, 256], F32)
                nc.vector.tensor_scalar_mul(mu[:], mu_ps[:], 1.0 / 64)
                sq = sw.tile([64, 256], F32)
                nc.vector.tensor_mul(sq[:], semb[:], semb[:])
                ms_ps = sps.tile([1, 256], F32, space="PSUM")
                nc.tensor.matmul(ms_ps[:], ones64[:], sq[:], start=True, stop=True)
                msq = sw.tile([1, 256], F32)
                nc.vector.tensor_scalar_mul(msq[:], ms_ps[:], 1.0 / 64)
                mu2 = sw.tile([1, 256], F32)
                nc.vector.tensor_mul(mu2[:], mu[:], mu[:])
                var = sw.tile([1, 256], F32)
                nc.vector.tensor_tensor(var[:], msq[:], mu2[:], op=ALU.subtract)
                nc.vector.tensor_scalar_add(var[:], var[:], 1e-5)
                std = sw.tile([1, 256], F32)
                nc.scalar.sqrt(std[:], var[:])
                istd = sw.tile([1, 256], F32)
                nc.vector.reciprocal(istd[:], std[:])
                mub_ps = sps.tile([64, 256], F32, space="PSUM")
                nc.tensor.matmul(mub_ps[:], ones1w[:1, :64], mu[:], start=True, stop=True)
                ib_ps = sps.tile([64, 256], F32, space="PSUM")
                nc.tensor.matmul(ib_ps[:], ones1w[:1, :64], istd[:], start=True, stop=True)
                d8 = sw.tile([8, 256], F32)
                nc.vector.tensor_tensor(d8[:], semb[0:8, :], mub_ps[0:8, :], op=ALU.subtract)
                nc.vector.tensor_tensor(s2_sb[:, 0:256], d8[:], ib_ps[0:8, :], op=ALU.mult)
                nc.vector.tensor_copy(s2_sb[:, 256:512], s2_sb[:, 0:256])
                nc.sync.dma_start(s2d[:], s2_sb[:])

            # ================= embedding gather + transpose =================
            with (
                tc.tile_pool(name="gat", bufs=3) as gp,
                tc.tile_pool(name="gps", bufs=2, space="PSUM") as gpsm,
            ):
                for g in range(NT):
                    egath = gp.tile([128, 256], F32)
                    nc.gpsimd.indirect_dma_start(
                        out=egath[:], out_offset=None, in_=emb_e[:],
                        in_offset=IndirectOffsetOnAxis(ap=xp_sb[:, g:g + 1], axis=0))
                    for dc, eT in ((0, eT0), (1, eT1)):
                        tps = gpsm.tile([128, 128], F32, space="PSUM")
                        nc.tensor.transpose(tps[:], egath[:, dc * 128:(dc + 1) * 128], ident[:])
                        nc.vector.tensor_copy(eT[:, g * 128:(g + 1) * 128], tps[:])

            # ================= xs = e @ w_ih.T + b (transposed layout) ======
            xs_v = xsT[:].rearrange("p (t q) -> p t q", q=64)
            with tc.tile_pool(name="xps", bufs=2, space="PSUM") as xpsm:
                for nch in range(NCH):
                    for gb in range(8):
                        xps = xpsm.tile([128, 512], F32, space="PSUM")
                        nc.tensor.matmul(xps[:], wihT_sb[:, gb * 128:(gb + 1) * 128],
                                         eT0[:, nch * 512:(nch + 1) * 512],
                                         start=True, stop=False)
                        nc.tensor.matmul(xps[:], wihT_sb[:, 1024 + gb * 128:1024 + (gb + 1) * 128],
                                         eT1[:, nch * 512:(nch + 1) * 512],
                                         start=False, stop=True)
                        nc.scalar.activation(
                            xs_v[:, nch * 64:(nch + 1) * 64, gb * 8:(gb + 1) * 8],
                            xps[:], AF.Identity, bias=bl_sb[:, gb:gb + 1])

            # ================= LSTM recurrence ==============================
            nc.gpsimd.memset(hT[:, 0:16], 0.0)
            with (
                tc.tile_pool(name="rec", bufs=3) as rp,
                tc.tile_pool(name="cst", bufs=3) as cpp,
                tc.tile_pool(name="rps", bufs=2, space="PSUM") as rpsm,
            ):
                c_prev = cpp.tile([128, 16], F32)
                nc.vector.memset(c_prev[:], 0.0)
                for t in range(T_steps):
                    gps = rpsm.tile([128, 64], F32, space="PSUM")
                    for gb in range(8):
                        nc.tensor.matmul(
                            gps[:, gb * 8:(gb + 1) * 8], ident_bf[:],
                            xsT[:, t * 64 + gb * 8: t * 64 + (gb + 1) * 8],
                            start=(gb == 0), stop=False)
                    for gb in range(8):
                        for dj in range(2):
                            nc.tensor.matmul(
                                gps[:, gb * 8:(gb + 1) * 8],
                                whhT_sb[:, dj * 1024 + gb * 128: dj * 1024 + (gb + 1) * 128],
                                hT[:, t * 16 + dj * 8: t * 16 + (dj + 1) * 8],
                                start=False, stop=(dj == 1))
                    sig = rp.tile([128, 48], F32)
                    nc.scalar.activation(sig[:], gps[:, 0:48], AF.Sigmoid)
                    gg = rp.tile([128, 16], F32)
                    nc.scalar.activation(gg[:], gps[:, 48:64], AF.Tanh)
                    m1 = rp.tile([128, 16], F32)
                    nc.vector.tensor_mul(m1[:], sig[:, 16:32], c_prev[:])
                    m2 = rp.tile([128, 16], F32)
                    nc.vector.tensor_mul(m2[:], sig[:, 0:16], gg[:])
                    c_new = cpp.tile([128, 16], F32)
                    nc.vector.tensor_tensor(c_new[:], m1[:], m2[:], op=ALU.add)
                    thc = rp.tile([128, 16], F32)
                    nc.scalar.activation(thc[:], c_new[:], AF.Tanh)
                    nc.vector.tensor_mul(hT[:, (t + 1) * 16:(t + 2) * 16], sig[:, 32:48], thc[:])
                    c_prev = c_new

            # ================= attention ====================================
            hT_v = hT[:].rearrange("p (t dj b) -> p dj b t", dj=2, b=8)
            with (
                tc.tile_pool(name="att", bufs=2) as ap_,
                tc.tile_pool(name="atp", bufs=2, space="PSUM") as apsm,
                tc.tile_pool(name="scp", bufs=1, space="PSUM") as scpsm,
                tc.tile_pool(name="sm", bufs=1) as smp,
            ):
                sc_ps = scpsm.tile([8, T_steps], F32, space="PSUM")
                for b in range(8):
                    cw = ap_.tile([128, 384], BF16)
                    for j in range(3):
                        win = bass.AP(s2d[:].tensor, b * 512 + j * 128, [[1, 128], [1, 128]])
                        nc.sync.dma_start(cw[:, j * 128:(j + 1) * 128], win)
                    weff = ap_.tile([128, 512], BF16)
                    for mj in range(2):
                        wps = apsm.tile([128, 256], F32, space="PSUM")
                        for kc in range(2):
                            nc.tensor.matmul(wps[:], cw[:, (mj + kc) * 128:(mj + kc + 1) * 128],
                                             wy_sb[:, kc * 256:(kc + 1) * 256],
                                             start=(kc == 0), stop=(kc == 1))
                        nc.vector.tensor_copy(weff[:, mj * 256:(mj + 1) * 256], wps[:])
                    yt = ap_.tile([128, 2 * T_steps], BF16)
                    for ec in range(2):
                        yps = apsm.tile([128, T_steps], F32, space="PSUM")
                        for kc in range(2):
                            nc.tensor.matmul(yps[:], weff[:, kc * 256 + ec * 128: kc * 256 + (ec + 1) * 128],
                                             hT_v[:, kc, b, 1:T_steps + 1],
                                             start=(kc == 0), stop=(kc == 1))
                        nc.scalar.activation(yt[:, ec * T_steps:(ec + 1) * T_steps], yps[:], AF.Tanh)
                    for ec in range(2):
                        nc.tensor.matmul(sc_ps[:, 0:T_steps],
                                         wtoh_sb[:, ec * 64 + b * 8: ec * 64 + (b + 1) * 8],
                                         yt[:, ec * T_steps:(ec + 1) * T_steps],
                                         start=(b == 0 and ec == 0), stop=(b == 7 and ec == 1))
                # softmax over T (free axis)
                mx = smp.tile([8, 1], F32)
                nc.vector.tensor_reduce(mx[:], sc_ps[:, 0:T_steps], axis=mybir.AxisListType.X, op=ALU.max)
                nmx = smp.tile([8, 1], F32)
                nc.vector.tensor_scalar_mul(nmx[:], mx[:], -1.0)
                esc = smp.tile([8, T_steps], F32)
                ssum = smp.tile([8, 1], F32)
                nc.scalar.activation(esc[:], sc_ps[:, 0:T_steps], AF.Exp,
                                     bias=nmx[:, 0:1], accum_out=ssum[:, 0:1])
                rcs = smp.tile([8, 1], F32)
                nc.vector.reciprocal(rcs[:], ssum[:])
                nc.scalar.activation(a_sb[:], esc[:], AF.Copy, scale=rcs[:, 0:1])
                # r = sum_t a_t * h_t   (per b: broadcast a row, multiply, reduce)
                for b in range(8):
                    abc = apsm.tile([128, T_steps], F32, space="PSUM")
                    nc.tensor.matmul(abc[:, 0:T_steps], sel_sb[:, b * 128:(b + 1) * 128],
                                     a_sb[:], start=True, stop=True)
                    for dj in range(2):
                        wt_ = ap_.tile([128, T_steps], F32)
                        nc.vector.tensor_tensor(wt_[:, 0:T_steps], hT_v[:, dj, b, 1:T_steps + 1],
                                                abc[:, 0:T_steps], op=ALU.mult)
                        nc.vector.tensor_reduce(rT_sb[:, dj * 8 + b: dj * 8 + b + 1],
                                                wt_[:, 0:T_steps], axis=mybir.AxisListType.X,
                                                op=ALU.add)
                nc.vector.tensor_copy(rT_bf[:], rT_sb[:])

            # ================= final MLP + softmax ==========================
            with (
                tc.tile_pool(name="fin", bufs=1) as fp,
                tc.tile_pool(name="fps", bufs=1, space="PSUM") as fpsm,
            ):
                rr_ps = fpsm.tile([128, 16], F32, space="PSUM")
                hlast = hT[:, T_steps * 16:(T_steps + 1) * 16]
                for oc in range(2):
                    for kc in range(2):
                        nc.tensor.matmul(rr_ps[:, oc * 8:(oc + 1) * 8],
                                         wpT_sb[:, kc * 256 + oc * 128: kc * 256 + (oc + 1) * 128],
                                         rT_bf[:, kc * 8:(kc + 1) * 8],
                                         start=(kc == 0), stop=False)
                    for kc in range(2):
                        nc.tensor.matmul(rr_ps[:, oc * 8:(oc + 1) * 8],
                                         wxT_sb[:, kc * 256 + oc * 128: kc * 256 + (oc + 1) * 128],
                                         hlast[:, kc * 8:(kc + 1) * 8],
                                         start=False, stop=(kc == 1))
                rrT = fp.tile([128, 16], BF16)
                nc.scalar.activation(rrT[:], rr_ps[:], AF.Tanh)
                z_ps = fpsm.tile([128, 16], F32, space="PSUM")
                for oc in range(2):
                    for kc in range(2):
                        nc.tensor.matmul(z_ps[:, oc * 8:(oc + 1) * 8],
                                         wfT_sb[:, kc * 256 + oc * 128: kc * 256 + (oc + 1) * 128],
                                         rrT[:, kc * 8:(kc + 1) * 8],
                                         start=(kc == 0), stop=(kc == 1))
                e_sb = fp.tile([128, 16], F32)
                for oc in range(2):
                    nc.scalar.activation(e_sb[:, oc * 8:(oc + 1) * 8], z_ps[:, oc * 8:(oc + 1) * 8],
                                         AF.Exp, bias=bf_sb[:, oc:oc + 1])
                cs_ps = fpsm.tile([1, 16], F32, space="PSUM")
                nc.tensor.matmul(cs_ps[:], ones128[:], e_sb[:], start=True, stop=True)
                cs_sb = fp.tile([1, 16], F32)
                nc.vector.tensor_copy(cs_sb[:], cs_ps[:])
                s8 = fp.tile([1, 8], F32)
                nc.vector.tensor_tensor(s8[:], cs_sb[0:1, 0:8], cs_sb[0:1, 8:16], op=ALU.add)
                rc8 = fp.tile([1, 8], F32)
                nc.vector.reciprocal(rc8[:], s8[:])
                rc16 = fp.tile([1, 16], F32)
                nc.vector.tensor_copy(rc16[:, 0:8], rc8[:])
                nc.vector.tensor_copy(rc16[:, 8:16], rc8[:])
                rbc_ps = fpsm.tile([128, 16], F32, space="PSUM")
                nc.tensor.matmul(rbc_ps[:], ones1w[:], rc16[:], start=True, stop=True)
                yT_sb = fp.tile([128, 16], F32)
                nc.vector.tensor_tensor(yT_sb[:], e_sb[:], rbc_ps[:], op=ALU.mult)
                ytr_ps = fpsm.tile([16, 128], F32, space="PSUM")
                nc.tensor.transpose(ytr_ps[:], yT_sb[:], ident[:])
                ynat = fp.tile([16, 128], F32)
                nc.vector.tensor_copy(ynat[:], ytr_ps[:])
                for oc in range(2):
                    nc.sync.dma_start(out_e[0:8, oc * 128:(oc + 1) * 128],
                                      ynat[oc * 8:(oc + 1) * 8, :])

    nc.compile()
    return nc


_CACHE = {}


def _get_nc(T_steps=512):
    if T_steps not in _CACHE:
        _CACHE[T_steps] = build(T_steps)
    return _CACHE[T_steps]


def make_in_maps(x, s, emb, w_ih, w_hh, b_lstm, W_y, w_t, W_p, W_x, W_f, b_f,
                 T_steps=512):
    x = np.asarray(x).astype(np.int32)[:, :T_steps]
    s = np.asarray(s).astype(np.int32).reshape(64)
    emb = np.ascontiguousarray(np.asarray(emb, dtype=np.float32))
    wih_p = np.asarray(w_ih, dtype=np.float32)[_PERM]
    whh_p = np.asarray(w_hh, dtype=np.float32)[_PERM]
    bl_p = np.asarray(b_lstm, dtype=np.float32)[_PERM]

    def wt2sb(wT):  # [256, 1024] -> [128, 2048]
        return np.concatenate([wT[0:128], wT[128:256]], axis=1)

    wihT = wt2sb(wih_p.T).astype(bf16)
    whhT = wt2sb(whh_p.T).astype(bf16)
    bl_sb = bl_p.reshape(8, 128).T.copy().astype(np.float32)  # [128, 8]
    wy_sb = np.concatenate([np.asarray(W_y, np.float32)[0:128],
                            np.asarray(W_y, np.float32)[128:256]], axis=1).astype(bf16)
    w_t = np.asarray(w_t, np.float32)
    wtoh = np.zeros((128, 128), np.float32)
    for ec in range(2):
        for b in range(8):
            wtoh[:, ec * 64 + b * 8 + b] = w_t[ec * 128:(ec + 1) * 128]
    wtoh = wtoh.astype(bf16)
    sel = np.zeros((8, 1024), np.float32)
    for b in range(8):
        sel[b, b * 128:(b + 1) * 128] = 1.0
    sel = sel.astype(bf16)

    def t2sb(w):  # W [do, din] -> lhsT layout [128, 512] free=kc*256+do
        wT = np.asarray(w, np.float32).T  # [din, do]
        return np.concatenate([wT[0:128], wT[128:256]], axis=1).astype(bf16)

    wpT = t2sb(W_p)
    wxT = t2sb(W_x)
    wfT = t2sb(W_f)
    bf_sb = np.asarray(b_f, np.float32).reshape(2, 128).T.copy()

    common = dict(emb=emb, wihT=wihT, whhT=whhT, bl=bl_sb, wy=wy_sb, wtoh=wtoh,
                  sel=sel, wpT=wpT, wxT=wxT, wfT=wfT, bf=bf_sb)
    in_maps = []
    for c in range(NCORES):
        xs_c = x[c * BL:(c + 1) * BL]                      # [8, T]
        xflat = xs_c.T.reshape(-1)                         # t-major tokens
        xp = xflat.reshape(-1, 128).T.copy()               # [128, NT]
        sidx = np.roll(s, -BL * c).reshape(64, 1).copy()
        in_maps.append(dict(xp=xp, sidx=sidx, **common))
    return in_maps


def _install_trace_shim():
    """The agent image lacks antenv.axon_hooks; recreate it and install the
    ctypes NTFF hook from trn_boot so run_bass_kernel_spmd(trace=True) works."""
    import sys, types
    if "antenv.axon_hooks" not in sys.modules:
        mod = types.ModuleType("antenv.axon_hooks")
        mod._hook = None
        mod.set_axon_ntff_profile_hook = lambda h: setattr(mod, "_hook", h)
        mod.get_axon_ntff_profile_hook = lambda: mod._hook
        sys.modules["antenv.axon_hooks"] = mod
        import antenv
        antenv.axon_hooks = mod
    import antenv.axon_hooks as ah
    if ah.get_axon_ntff_profile_hook() is None:
        from trn_agent_boot.trn_boot import _ntff_profile_via_ctypes
        ah.set_axon_ntff_profile_hook(_ntff_profile_via_ctypes("/opt/axon/libaxon_pjrt.so"))
    import concourse.bass_utils as bu
    bu.upload_artifacts = lambda tmpdir: ""


def run(in_maps, T_steps=512, trace=False, tmpdir=None):
    nc = _get_nc(T_steps)
    if trace:
        _install_trace_shim()
    return run_bass_kernel_spmd(nc, in_maps, core_ids=list(range(NCORES)),
                                trace=trace, tmpdir=tmpdir)


def kernel(x, s, emb, w_ih, w_hh, b_lstm, W_y, w_t, W_p, W_x, W_f, b_f):
    in_maps = make_in_maps(x, s, emb, w_ih, w_hh, b_lstm, W_y, w_t, W_p, W_x,
                           W_f, b_f)
    res = run(in_maps)
    return np.concatenate([res.results[i]["out"] for i in range(NCORES)], axis=0)
